# revision 4
# baseline (speedup 1.0000x reference)
"""Trainium2 Bass kernel for single-head 2D attention (B=16, C=512, H=W=32).

Data-parallel over batch: 16 items / 8 cores = 2 per core; weights replicated.
All matmuls run in fp8 e4m3 with DoubleRow perf mode (two 128-deep contraction
slabs per PE instruction; measured ~45 ns per 512-wide matmul on HW, ~4.7x the
fp32r rate). Precision (fp64-referenced sim: rel ~5e-3 vs the 2e-2 gate) is
preserved by folding the projections on the host:

  * G-trick:  scores  s[j,i] = k_j . q_i = (G x_j) . x_i with G = wq^T wk
              -- the Q projection never runs on device.
  * W2-trick: out_raw[c',i] = sum_j est[j,i] * (W2 x_j)[c'] with W2 = wo wv
              -- the output projection never runs on device.
  * bias folds: q/k biases dropped (softmax-invariant + O(0.01) terms,
              8e-4 rel); bv/bo fold into the residual x' = x + bo + wo bv.

With the PE this fast the kernel is Act/DVE-bound (PSUM drains + exp + the
recip-scaled output), so the two batch items are software-pipelined:

    ... B(t): scores+den | A(t+1): kp/vp projections | C(t): out_raw+y ...

which keeps Act (est exps + kp drains) and DVE (vp drains, recip, y-mul)
saturated while the PE sprints between drain waits. The residual add runs on
the otherwise-idle GPSIMD/Pool engine (SBUF-only op, all-bf16).
"""

import math

import numpy as np

import concourse.mybir as mybir
import concourse.tile as tile
from concourse import bacc, bass_utils

B, C, H, W = 16, 512, 32, 32
N = H * W           # 1024 tokens
NCORES = 8
BPC = B // NCORES   # batch items per core
P = 128
CO = C // P         # 4 channel chunks
NB = N // 512       # 2 psum-bank slices of the token dim
NT = N // P         # 8 token chunks

_CACHE: dict = {}


def _build(reps: int = 1):
    f32 = mybir.dt.float32
    f8 = mybir.dt.float8e4
    bf16 = mybir.dt.bfloat16
    DR = mybir.MatmulPerfMode.DoubleRow
    Copy = mybir.ActivationFunctionType.Copy
    Exp = mybir.ActivationFunctionType.Exp

    nc = bacc.Bacc("TRN2", debug=False, enable_asserts=False, num_devices=NCORES)
    x8_d = nc.dram_tensor("x8", (BPC, C, N), f8, kind="ExternalInput").ap()
    xr_d = nc.dram_tensor("xr", (BPC, C, N), bf16, kind="ExternalInput").ap()
    gt_d = nc.dram_tensor("gt", (C, C), f8, kind="ExternalInput").ap()
    w2t_d = nc.dram_tensor("w2t", (C, C), f8, kind="ExternalInput").ap()
    ones_d = nc.dram_tensor("ones", (P, 2, P), f8, kind="ExternalInput").ap()
    y_d = nc.dram_tensor("y", (BPC, C, N), bf16, kind="ExternalOutput").ap()

    inv_sqrt_c = 1.0 / math.sqrt(C)

    with tile.TileContext(nc) as tc:
        with (
            tc.tile_pool(name="wp", bufs=1) as wp,
            tc.tile_pool(name="kpp", bufs=2) as kpp,
            tc.tile_pool(name="vpp", bufs=2) as vpp,
            tc.tile_pool(name="epp", bufs=2) as epp,
            tc.tile_pool(name="rpp", bufs=2) as rpp,
            tc.tile_pool(name="tp", bufs=4) as tp,
            tc.tile_pool(name="yp", bufs=4) as yp,
            tc.tile_pool(name="ps", bufs=3, space="PSUM") as ps,
            tc.tile_pool(name="dp", bufs=1, space="PSUM") as dp,
        ):
            ebias_t = wp.tile([P, 1], f32, tag="ebias")
            nc.vector.memset(ebias_t[:], -3.0)
            gt_t = wp.tile([P, CO, C], f8, tag="gt")
            w2t_t = wp.tile([P, CO, C], f8, tag="w2t")
            ones_t = wp.tile([P, 2, P], f8, tag="ones")
            x8_tiles = [
                wp.tile([P, CO, NB, 512], f8, tag=f"x8_{b}", name=f"x8_{b}")
                for b in range(BPC)
            ]
            xr_tiles = [
                wp.tile([P, CO, NB, 512], bf16, tag=f"xr_{b}", name=f"xr_{b}")
                for b in range(BPC)
            ]
            gt_r = gt_d.rearrange("(ci p) o -> p ci o", p=P)
            w2t_r = w2t_d.rearrange("(ci p) o -> p ci o", p=P)
            x8_r = [
                x8_d[b].rearrange("(ci p) (nb n) -> p ci nb n", p=P, nb=NB)
                for b in range(BPC)
            ]
            xr_r = [
                xr_d[b].rearrange("(ci p) (nb n) -> p ci nb n", p=P, nb=NB)
                for b in range(BPC)
            ]
            nc.sync.dma_start(gt_t[:], gt_r)
            for b in range(BPC):
                nc.sync.dma_start(x8_tiles[b][:], x8_r[b])
            nc.sync.dma_start(w2t_t[:], w2t_r)
            nc.sync.dma_start(ones_t[:], ones_d)
            for b in range(BPC):
                nc.sync.dma_start(xr_tiles[b][:], xr_r[b])

            def emit_A(b):
                """kp = G x (channel-major) and vp = (W2 x)^T (token-major)."""
                x8_t = x8_tiles[b]
                kp = kpp.tile([P, CO, NB, 512], f8, tag="kp")
                for oc in range(CO):
                    pt = ps.tile([P, NB, 512], f32, tag="ps", name="kp_pt")
                    for nb in range(NB):
                        for cip in range(0, CO, 2):
                            nc.tensor.matmul(
                                pt[:, nb],
                                gt_t[:, cip:cip + 2, oc * P:(oc + 1) * P],
                                x8_t[:, cip:cip + 2, nb],
                                start=(cip == 0), stop=(cip == CO - 2),
                                perf_mode=DR,
                            )
                    if oc == 0:
                        nc.vector.tensor_copy(kp[:, oc], pt[:])
                    else:
                        nc.scalar.activation(kp[:, oc], pt[:], Copy)
                vp = vpp.tile([P, NT, C], f8, tag="vp")
                for t8p in range(0, NT, 2):
                    pt = ps.tile([P, 2, 512], f32, tag="ps", name="vp_pt")
                    for s in range(2):
                        t8 = t8p + s
                        for cip in range(0, CO, 2):
                            nc.tensor.matmul(
                                pt[:, s],
                                x8_t[:, cip:cip + 2, t8 // 4,
                                     (t8 % 4) * P:(t8 % 4 + 1) * P],
                                w2t_t[:, cip:cip + 2, :],
                                start=(cip == 0), stop=(cip == CO - 2),
                                perf_mode=DR,
                            )
                    nc.vector.tensor_copy(vp[:, t8p:t8p + 2, :], pt[:])
                return kp, vp

            def emit_B(b, kp):
                """s = kp^T x ; est = exp(s/sqrt(C)-3) fp8; den via all-ones
                DoubleRow matmuls as est pairs land; recip on DVE."""
                x8_t = x8_tiles[b]
                est = epp.tile([P, NT, NB, 512], f8, tag="est")
                den_pt = dp.tile([P, NB, 512], f32, tag="den")
                for jc in range(NT):
                    pt = ps.tile([P, NB, 512], f32, tag="ps", name="sc_pt")
                    for ib in range(NB):
                        for ocp in range(0, CO, 2):
                            nc.tensor.matmul(
                                pt[:, ib],
                                kp[:, ocp:ocp + 2, jc // 4,
                                   (jc % 4) * P:(jc % 4 + 1) * P],
                                x8_t[:, ocp:ocp + 2, ib],
                                start=(ocp == 0), stop=(ocp == CO - 2),
                                perf_mode=DR,
                            )
                    nc.scalar.activation(est[:, jc], pt[:], Exp,
                                         bias=ebias_t[:], scale=inv_sqrt_c)
                    if jc % 2 == 1:
                        for ib in range(NB):
                            nc.tensor.matmul(
                                den_pt[:, ib],
                                ones_t[:],
                                est[:, jc - 1:jc + 1, ib, :],
                                start=(jc == 1), stop=(jc == NT - 1),
                                perf_mode=DR,
                            )
                recip = rpp.tile([P, NB, 512], f32, tag="recip")
                nc.vector.reciprocal(recip[:], den_pt[:])
                return est, recip

            def emit_C(b, vp, est, recip):
                """out_raw = vp^T est ; y = out_raw*recip + x' -> DRAM."""
                xr_t = xr_tiles[b]
                for cc in range(CO):
                    pt = ps.tile([P, NB, 512], f32, tag="ps", name="or_pt")
                    for ib in range(NB):
                        for jcp in range(0, NT, 2):
                            nc.tensor.matmul(
                                pt[:, ib],
                                vp[:, jcp:jcp + 2, cc * P:(cc + 1) * P],
                                est[:, jcp:jcp + 2, ib, :],
                                start=(jcp == 0), stop=(jcp == NT - 2),
                                perf_mode=DR,
                            )
                    t = tp.tile([P, NB, 512], bf16, tag="t")
                    nc.vector.tensor_mul(t[:], pt[:], recip[:])
                    # all-bf16 SBUF add on the otherwise idle GPSIMD engine
                    yt = yp.tile([P, NB, 512], bf16, tag="y")
                    nc.gpsimd.tensor_add(yt[:], t[:], xr_t[:, cc])
                    nc.sync.dma_start(
                        y_d[b, cc * P:(cc + 1) * P, :], yt[:])

            items = [i for _ in range(reps) for i in range(BPC)]
            kp_vp = emit_A(items[0])
            for ti, b in enumerate(items):
                kp, vp = kp_vp
                est, recip = emit_B(b, kp)
                if ti + 1 < len(items):
                    kp_vp = emit_A(items[ti + 1])
                emit_C(b, vp, est, recip)
    nc.compile()
    return nc


def _prep_inputs(inputs):
    f8np = mybir.dt.np(mybir.dt.float8e4)
    bf16np = mybir.dt.np(mybir.dt.bfloat16)

    def q8(a):
        return np.clip(a, -240.0, 240.0).astype(f8np)

    x = np.asarray(inputs["x"], np.float32).reshape(B, C, N)
    wq = np.asarray(inputs["wq"], np.float64)
    wk = np.asarray(inputs["wk"], np.float64)
    wv = np.asarray(inputs["wv"], np.float64)
    wo = np.asarray(inputs["wo"], np.float64)
    bv = np.asarray(inputs["bv"], np.float64)
    bo = np.asarray(inputs["bo"], np.float64)

    G = (wq.T @ wk).astype(np.float32)      # s[j,i] = (G x_j) . x_i
    W2 = (wo @ wv).astype(np.float32)       # v'_j = W2 x_j
    bo_eff = (bo + wo @ bv).astype(np.float32)

    # lhsT layouts: gt[ci*P+p, o] = G[o, ci*P+p]; same for W2.
    shared = {
        "gt": np.ascontiguousarray(q8(G.T)),
        "w2t": np.ascontiguousarray(q8(W2.T)),
        "ones": np.ones((P, 2, P), f8np),
    }
    xr = (x + bo_eff[None, :, None]).astype(bf16np)
    in_maps = [
        {
            **shared,
            "x8": np.ascontiguousarray(q8(x[i * BPC:(i + 1) * BPC])),
            "xr": np.ascontiguousarray(xr[i * BPC:(i + 1) * BPC]),
        }
        for i in range(NCORES)
    ]
    return in_maps


def _make_axon_runner(nc):
    """Cached jitted shard_map runner for the axon/PJRT path."""
    import jax
    from jax.sharding import Mesh, NamedSharding, PartitionSpec

    import warnings

    with warnings.catch_warnings():
        warnings.simplefilter("ignore")
        from jax.experimental.shard_map import shard_map

    import concourse.bass2jax as b2j

    b2j.install_neuronx_cc_hook()
    partition_name = nc.partition_id_tensor.name if nc.partition_id_tensor else None
    in_names, out_names, out_avals = [], [], []
    for alloc in nc.m.functions[0].allocations:
        if not isinstance(alloc, mybir.MemoryLocationSet):
            continue
        name = alloc.memorylocations[0].name
        if alloc.kind == "ExternalInput":
            if name != partition_name:
                in_names.append(name)
        elif alloc.kind == "ExternalOutput":
            out_names.append(name)
            out_avals.append(
                jax.core.ShapedArray(tuple(alloc.tensor_shape),
                                     mybir.dt.np(alloc.dtype)))
    n_params = len(in_names)
    bind_in_names = list(in_names) + list(out_names)
    if partition_name is not None:
        bind_in_names.append(partition_name)

    def _body(*args):
        operands = list(args)
        if partition_name is not None:
            operands.append(b2j.partition_id_tensor())
        return tuple(b2j._bass_exec_p.bind(
            *operands,
            out_avals=tuple(out_avals),
            in_names=tuple(bind_in_names),
            out_names=tuple(out_names),
            lowering_input_output_aliases=(),
            sim_require_finite=True,
            sim_require_nnan=True,
            nc=nc,
        ))

    devices = jax.devices()[:NCORES]
    mesh = Mesh(np.asarray(devices), ("core",))
    n_outs = len(out_avals)
    fn = jax.jit(
        shard_map(_body, mesh=mesh,
                  in_specs=(PartitionSpec("core"),) * (n_params + n_outs),
                  out_specs=(PartitionSpec("core"),) * n_outs,
                  check_rep=False),
        keep_unused=True,
    )
    sharding = NamedSharding(mesh, PartitionSpec("core"))
    dev_zeros = [
        jax.device_put(
            np.zeros((NCORES * a.shape[0], *a.shape[1:]), a.dtype), sharding)
        for a in out_avals
    ]

    def run(in_maps):
        concat_in = [
            np.concatenate([np.asarray(m[nm]) for m in in_maps], axis=0)
            for nm in in_names
        ]
        dev_in = [jax.device_put(a, sharding) for a in concat_in]
        outs = fn(*dev_in, *dev_zeros)
        return np.asarray(outs[0])

    return run


def kernel(**inputs) -> np.ndarray:
    if "nc" not in _CACHE:
        _CACHE["nc"] = _build()
    nc = _CACHE["nc"]
    in_maps = _prep_inputs(inputs)

    from concourse._compat import axon_active

    if axon_active():
        if "runner" not in _CACHE:
            _CACHE["runner"] = _make_axon_runner(nc)
        y = _CACHE["runner"](in_maps).reshape(B, C, N)
    else:
        results = bass_utils.run_bass_kernel_spmd(
            nc, in_maps, core_ids=list(range(NCORES))).results
        y = np.concatenate([r["y"] for r in results], axis=0).reshape(B, C, N)
    return y.reshape(B, C, H, W).astype(np.float32)


# revision 5
# speedup vs baseline: 1.0067x; 1.0067x over previous
"""Trainium2 Bass kernel for single-head 2D attention (B=16, C=512, H=W=32).

Data-parallel over batch: 16 items / 8 cores = 2 per core; weights replicated.
All matmuls run in fp8 e4m3 with DoubleRow perf mode (two 128-deep contraction
slabs per PE instruction; measured ~45 ns per 512-wide matmul on HW, ~4.7x the
fp32r rate). Precision (fp64-referenced sim: rel ~5e-3 vs the 2e-2 gate) is
preserved by folding the projections on the host:

  * G-trick:  scores  s[j,i] = k_j . q_i = (G x_j) . x_i with G = wq^T wk
              -- the Q projection never runs on device.
  * W2-trick: out_raw[c',i] = sum_j est[j,i] * (W2 x_j)[c'] with W2 = wo wv
              -- the output projection never runs on device.
  * bias folds: q/k biases dropped (softmax-invariant + O(0.01) terms,
              8e-4 rel); bv/bo fold into the residual x' = x + bo + wo bv.

With the PE this fast the kernel is Act/DVE-bound (PSUM drains + exp + the
recip-scaled output), so the two batch items are software-pipelined:

    ... B(t): scores+den | A(t+1): kp/vp projections | C(t): out_raw+y ...

which keeps Act (est exps + kp drains) and DVE (vp drains, recip, y-mul)
saturated while the PE sprints between drain waits. The residual add runs on
the otherwise-idle GPSIMD/Pool engine (SBUF-only op, all-bf16).
"""

import math

import numpy as np

import concourse.mybir as mybir
import concourse.tile as tile
from concourse import bacc, bass_utils

B, C, H, W = 16, 512, 32, 32
N = H * W           # 1024 tokens
NCORES = 8
BPC = B // NCORES   # batch items per core
P = 128
CO = C // P         # 4 channel chunks
NB = N // 512       # 2 psum-bank slices of the token dim
NT = N // P         # 8 token chunks

_CACHE: dict = {}


def _build(reps: int = 1):
    f32 = mybir.dt.float32
    f8 = mybir.dt.float8e4
    bf16 = mybir.dt.bfloat16
    DR = mybir.MatmulPerfMode.DoubleRow
    Ident = mybir.ActivationFunctionType.Identity
    Exp = mybir.ActivationFunctionType.Exp

    nc = bacc.Bacc("TRN2", debug=False, enable_asserts=False, num_devices=NCORES)
    x8_d = nc.dram_tensor("x8", (BPC, C, N), f8, kind="ExternalInput").ap()
    xr_d = nc.dram_tensor("xr", (BPC, C, N), bf16, kind="ExternalInput").ap()
    gt_d = nc.dram_tensor("gt", (C, C), f8, kind="ExternalInput").ap()
    w2t_d = nc.dram_tensor("w2t", (C, C), f8, kind="ExternalInput").ap()
    ones_d = nc.dram_tensor("ones", (P, 2, P), f8, kind="ExternalInput").ap()
    y_d = nc.dram_tensor("y", (BPC, C, N), bf16, kind="ExternalOutput").ap()

    inv_sqrt_c = 1.0 / math.sqrt(C)

    with tile.TileContext(nc) as tc:
        with (
            tc.tile_pool(name="wp", bufs=1) as wp,
            tc.tile_pool(name="kpp", bufs=2) as kpp,
            tc.tile_pool(name="vpp", bufs=2) as vpp,
            tc.tile_pool(name="epp", bufs=2) as epp,
            tc.tile_pool(name="rpp", bufs=2) as rpp,
            tc.tile_pool(name="tp", bufs=4) as tp,
            tc.tile_pool(name="yp", bufs=4) as yp,
            tc.tile_pool(name="ps", bufs=3, space="PSUM") as ps,
            tc.tile_pool(name="dp", bufs=1, space="PSUM") as dp,
            tc.tile_pool(name="esp", bufs=2) as esp,
        ):
            ebias_t = wp.tile([P, 1], f32, tag="ebias")
            nc.vector.memset(ebias_t[:], -3.0)
            gt_t = wp.tile([P, CO, C], f8, tag="gt")
            w2t_t = wp.tile([P, CO, C], f8, tag="w2t")
            ones_t = wp.tile([P, 2, P], f8, tag="ones")
            zbias_t = wp.tile([P, 1], f32, tag="zbias")
            nc.vector.memset(zbias_t[:], 0.0)
            x8_tiles = [
                wp.tile([P, CO, NB, 512], f8, tag=f"x8_{b}", name=f"x8_{b}")
                for b in range(BPC)
            ]
            xr_tiles = [
                wp.tile([P, CO, NB, 512], bf16, tag=f"xr_{b}", name=f"xr_{b}")
                for b in range(BPC)
            ]
            gt_r = gt_d.rearrange("(ci p) o -> p ci o", p=P)
            w2t_r = w2t_d.rearrange("(ci p) o -> p ci o", p=P)
            x8_r = [
                x8_d[b].rearrange("(ci p) (nb n) -> p ci nb n", p=P, nb=NB)
                for b in range(BPC)
            ]
            xr_r = [
                xr_d[b].rearrange("(ci p) (nb n) -> p ci nb n", p=P, nb=NB)
                for b in range(BPC)
            ]
            nc.sync.dma_start(gt_t[:], gt_r)
            for b in range(BPC):
                nc.sync.dma_start(x8_tiles[b][:], x8_r[b])
            nc.sync.dma_start(w2t_t[:], w2t_r)
            nc.sync.dma_start(ones_t[:], ones_d)
            for b in range(BPC):
                nc.sync.dma_start(xr_tiles[b][:], xr_r[b])

            def emit_A(b):
                """kp = G x (channel-major) and vp = (W2 x)^T (token-major)."""
                x8_t = x8_tiles[b]
                kp = kpp.tile([P, CO, NB, 512], f8, tag="kp")
                for oc in range(CO):
                    pt = ps.tile([P, NB, 512], f32, tag="ps", name="kp_pt")
                    for nb in range(NB):
                        for cip in range(0, CO, 2):
                            nc.tensor.matmul(
                                pt[:, nb],
                                gt_t[:, cip:cip + 2, oc * P:(oc + 1) * P],
                                x8_t[:, cip:cip + 2, nb],
                                start=(cip == 0), stop=(cip == CO - 2),
                                perf_mode=DR,
                            )
                    nc.scalar.activation(kp[:, oc], pt[:], Ident,
                                         bias=zbias_t[:])
                vp = vpp.tile([P, NT, C], f8, tag="vp")
                for t8p in range(0, NT, 2):
                    pt = ps.tile([P, 2, 512], f32, tag="ps", name="vp_pt")
                    for s in range(2):
                        t8 = t8p + s
                        for cip in range(0, CO, 2):
                            nc.tensor.matmul(
                                pt[:, s],
                                x8_t[:, cip:cip + 2, t8 // 4,
                                     (t8 % 4) * P:(t8 % 4 + 1) * P],
                                w2t_t[:, cip:cip + 2, :],
                                start=(cip == 0), stop=(cip == CO - 2),
                                perf_mode=DR,
                            )
                    nc.vector.tensor_copy(vp[:, t8p:t8p + 2, :], pt[:])
                return kp, vp

            def emit_B(b, kp):
                """s = kp^T x ; est = exp(s/sqrt(C)-3) fp8. The chunk sum
                esum accumulates on DVE as est tiles land (keeps the den
                work off the bottleneck PE); the cross-partition reduction
                is two cheap plain-fp8 ones-matmuls emitted by the caller."""
                x8_t = x8_tiles[b]
                est = epp.tile([P, NT, NB, 512], f8, tag="est")
                esum = esp.tile([P, 2, NB, 512], f32, tag="esum")
                esum8 = esp.tile([P, 2, NB, 512], f8, tag="esum8")
                for jc in range(NT):
                    pt = ps.tile([P, NB, 512], f32, tag="ps", name="sc_pt")
                    for ib in range(NB):
                        for ocp in range(0, CO, 2):
                            nc.tensor.matmul(
                                pt[:, ib],
                                kp[:, ocp:ocp + 2, jc // 4,
                                   (jc % 4) * P:(jc % 4 + 1) * P],
                                x8_t[:, ocp:ocp + 2, ib],
                                start=(ocp == 0), stop=(ocp == CO - 2),
                                perf_mode=DR,
                            )
                    nc.scalar.activation(est[:, jc], pt[:], Exp,
                                         bias=ebias_t[:], scale=inv_sqrt_c)
                    # two half-range chunk sums (fp8-safe magnitudes), each
                    # finishing in an fp8 tile for the ones-DoubleRow den
                    h, r = jc // 4, jc % 4
                    if r == 1:
                        nc.vector.tensor_add(esum[:, h], est[:, jc - 1],
                                             est[:, jc])
                    elif r == 2:
                        nc.vector.tensor_add(esum[:, h], esum[:, h],
                                             est[:, jc])
                    elif r == 3:
                        nc.vector.tensor_add(esum8[:, h], esum[:, h],
                                             est[:, jc])
                return est, esum8

            def emit_C(b, vp, est, esum8):
                """out_raw = vp^T est ; y = out_raw*recip + x' -> DRAM.
                den = cross-partition sum of esum8 via plain-fp8 ones
                matmuls, slotted between the first out_raw groups."""
                xr_t = xr_tiles[b]
                recip = rpp.tile([P, NB, 512], f32, tag="recip")
                for cc in range(CO):
                    pt = ps.tile([P, NB, 512], f32, tag="ps", name="or_pt")
                    for ib in range(NB):
                        for jcp in range(0, NT, 2):
                            nc.tensor.matmul(
                                pt[:, ib],
                                vp[:, jcp:jcp + 2, cc * P:(cc + 1) * P],
                                est[:, jcp:jcp + 2, ib, :],
                                start=(jcp == 0), stop=(jcp == NT - 2),
                                perf_mode=DR,
                            )
                    if cc == 0:
                        den_pt = dp.tile([P, NB, 512], f32, tag="den")
                        for ib in range(NB):
                            nc.tensor.matmul(den_pt[:, ib], ones_t[:],
                                             esum8[:, :, ib, :], start=True,
                                             stop=True, perf_mode=DR)
                        nc.vector.reciprocal(recip[:], den_pt[:])
                    t = tp.tile([P, NB, 512], bf16, tag="t")
                    nc.vector.tensor_mul(t[:], pt[:], recip[:])
                    # all-bf16 SBUF add on the otherwise idle GPSIMD engine
                    yt = yp.tile([P, NB, 512], bf16, tag="y")
                    nc.gpsimd.tensor_add(yt[:], t[:], xr_t[:, cc])
                    nc.sync.dma_start(
                        y_d[b, cc * P:(cc + 1) * P, :], yt[:])

            items = [i for _ in range(reps) for i in range(BPC)]
            kp_vp = emit_A(items[0])
            for ti, b in enumerate(items):
                kp, vp = kp_vp
                est, esum8 = emit_B(b, kp)
                if ti + 1 < len(items):
                    kp_vp = emit_A(items[ti + 1])
                emit_C(b, vp, est, esum8)
    nc.compile()
    return nc


def _prep_inputs(inputs):
    f8np = mybir.dt.np(mybir.dt.float8e4)
    bf16np = mybir.dt.np(mybir.dt.bfloat16)

    def q8(a):
        return np.clip(a, -240.0, 240.0).astype(f8np)

    x = np.asarray(inputs["x"], np.float32).reshape(B, C, N)
    wq = np.asarray(inputs["wq"], np.float64)
    wk = np.asarray(inputs["wk"], np.float64)
    wv = np.asarray(inputs["wv"], np.float64)
    wo = np.asarray(inputs["wo"], np.float64)
    bv = np.asarray(inputs["bv"], np.float64)
    bo = np.asarray(inputs["bo"], np.float64)

    G = (wq.T @ wk).astype(np.float32)      # s[j,i] = (G x_j) . x_i
    W2 = (wo @ wv).astype(np.float32)       # v'_j = W2 x_j
    bo_eff = (bo + wo @ bv).astype(np.float32)

    # lhsT layouts: gt[ci*P+p, o] = G[o, ci*P+p]; same for W2.
    shared = {
        "gt": np.ascontiguousarray(q8(G.T)),
        "w2t": np.ascontiguousarray(q8(W2.T)),
        "ones": np.ones((P, 2, P), f8np),
    }
    xr = (x + bo_eff[None, :, None]).astype(bf16np)
    in_maps = [
        {
            **shared,
            "x8": np.ascontiguousarray(q8(x[i * BPC:(i + 1) * BPC])),
            "xr": np.ascontiguousarray(xr[i * BPC:(i + 1) * BPC]),
        }
        for i in range(NCORES)
    ]
    return in_maps


def _make_axon_runner(nc):
    """Cached jitted shard_map runner for the axon/PJRT path."""
    import jax
    from jax.sharding import Mesh, NamedSharding, PartitionSpec

    import warnings

    with warnings.catch_warnings():
        warnings.simplefilter("ignore")
        from jax.experimental.shard_map import shard_map

    import concourse.bass2jax as b2j

    b2j.install_neuronx_cc_hook()
    partition_name = nc.partition_id_tensor.name if nc.partition_id_tensor else None
    in_names, out_names, out_avals = [], [], []
    for alloc in nc.m.functions[0].allocations:
        if not isinstance(alloc, mybir.MemoryLocationSet):
            continue
        name = alloc.memorylocations[0].name
        if alloc.kind == "ExternalInput":
            if name != partition_name:
                in_names.append(name)
        elif alloc.kind == "ExternalOutput":
            out_names.append(name)
            out_avals.append(
                jax.core.ShapedArray(tuple(alloc.tensor_shape),
                                     mybir.dt.np(alloc.dtype)))
    n_params = len(in_names)
    bind_in_names = list(in_names) + list(out_names)
    if partition_name is not None:
        bind_in_names.append(partition_name)

    def _body(*args):
        operands = list(args)
        if partition_name is not None:
            operands.append(b2j.partition_id_tensor())
        return tuple(b2j._bass_exec_p.bind(
            *operands,
            out_avals=tuple(out_avals),
            in_names=tuple(bind_in_names),
            out_names=tuple(out_names),
            lowering_input_output_aliases=(),
            sim_require_finite=True,
            sim_require_nnan=True,
            nc=nc,
        ))

    devices = jax.devices()[:NCORES]
    mesh = Mesh(np.asarray(devices), ("core",))
    n_outs = len(out_avals)
    fn = jax.jit(
        shard_map(_body, mesh=mesh,
                  in_specs=(PartitionSpec("core"),) * (n_params + n_outs),
                  out_specs=(PartitionSpec("core"),) * n_outs,
                  check_rep=False),
        keep_unused=True,
    )
    sharding = NamedSharding(mesh, PartitionSpec("core"))
    dev_zeros = [
        jax.device_put(
            np.zeros((NCORES * a.shape[0], *a.shape[1:]), a.dtype), sharding)
        for a in out_avals
    ]

    def run(in_maps):
        concat_in = [
            np.concatenate([np.asarray(m[nm]) for m in in_maps], axis=0)
            for nm in in_names
        ]
        dev_in = [jax.device_put(a, sharding) for a in concat_in]
        outs = fn(*dev_in, *dev_zeros)
        return np.asarray(outs[0])

    return run


def kernel(**inputs) -> np.ndarray:
    if "nc" not in _CACHE:
        _CACHE["nc"] = _build()
    nc = _CACHE["nc"]
    in_maps = _prep_inputs(inputs)

    from concourse._compat import axon_active

    if axon_active():
        if "runner" not in _CACHE:
            _CACHE["runner"] = _make_axon_runner(nc)
        y = _CACHE["runner"](in_maps).reshape(B, C, N)
    else:
        results = bass_utils.run_bass_kernel_spmd(
            nc, in_maps, core_ids=list(range(NCORES))).results
        y = np.concatenate([r["y"] for r in results], axis=0).reshape(B, C, N)
    return y.reshape(B, C, H, W).astype(np.float32)


# revision 6
# speedup vs baseline: 1.8253x; 1.8131x over previous
"""Trainium2 Bass kernel for single-head 2D attention (B=16, C=512, H=W=32).

Data-parallel over batch: 16 items / 8 cores = 2 per core; weights replicated.
All matmuls run in fp8 e4m3 with DoubleRow perf mode (two 128-deep contraction
slabs per PE instruction; measured ~45 ns per 512-wide matmul on HW, ~4.7x the
fp32r rate). Precision (fp64-referenced sim: rel ~5e-3 vs the 2e-2 gate) is
preserved by folding the projections on the host:

  * G-trick:  scores  s[j,i] = k_j . q_i = (G x_j) . x_i with G = wq^T wk
              -- the Q projection never runs on device.
  * W2-trick: out_raw[c',i] = sum_j est[j,i] * (W2 x_j)[c'] with W2 = wo wv
              -- the output projection never runs on device.
  * bias folds: q/k biases dropped (softmax-invariant + O(0.01) terms,
              8e-4 rel); bv/bo fold into the residual x' = x + bo + wo bv.

With the PE this fast the kernel is Act/DVE-bound (PSUM drains + exp + the
recip-scaled output), so the two batch items are software-pipelined:

    ... B(t): scores+den | A(t+1): kp/vp projections | C(t): out_raw+y ...

which keeps Act (est exps + kp drains) and DVE (vp drains, recip, y-mul)
saturated while the PE sprints between drain waits. The residual add runs on
the otherwise-idle GPSIMD/Pool engine (SBUF-only op, all-bf16).
"""

import math

import numpy as np

import concourse.mybir as mybir
import concourse.tile as tile
from concourse import bacc, bass_utils

B, C, H, W = 16, 512, 32, 32
N = H * W           # 1024 tokens
NCORES = 8
BPC = B // NCORES   # batch items per core
P = 128
CO = C // P         # 4 channel chunks
NB = N // 512       # 2 psum-bank slices of the token dim
NT = N // P         # 8 token chunks

_CACHE: dict = {}


def _build(reps: int = 1):
    f32 = mybir.dt.float32
    f8 = mybir.dt.float8e4
    bf16 = mybir.dt.bfloat16
    DR = mybir.MatmulPerfMode.DoubleRow
    Ident = mybir.ActivationFunctionType.Identity
    Exp = mybir.ActivationFunctionType.Exp

    nc = bacc.Bacc("TRN2", debug=False, enable_asserts=False, num_devices=NCORES)
    x8_d = nc.dram_tensor("x8", (BPC, C, N), f8, kind="ExternalInput").ap()
    xr_d = nc.dram_tensor("xr", (BPC, C, N), bf16, kind="ExternalInput").ap()
    gt_d = nc.dram_tensor("gt", (C, C), f8, kind="ExternalInput").ap()
    w2t_d = nc.dram_tensor("w2t", (C, C), f8, kind="ExternalInput").ap()
    ones_d = nc.dram_tensor("ones", (P, 2, P), f8, kind="ExternalInput").ap()
    y_d = nc.dram_tensor("y", (BPC, C, N), bf16, kind="ExternalOutput").ap()

    inv_sqrt_c = 1.0 / math.sqrt(C)

    with tile.TileContext(nc) as tc:
        with (
            tc.tile_pool(name="wp", bufs=1) as wp,
            tc.tile_pool(name="kpp", bufs=2) as kpp,
            tc.tile_pool(name="vpp", bufs=2) as vpp,
            tc.tile_pool(name="epp", bufs=2) as epp,
            tc.tile_pool(name="rpp", bufs=2) as rpp,
            tc.tile_pool(name="tp", bufs=4) as tp,
            tc.tile_pool(name="yp", bufs=4) as yp,
            tc.tile_pool(name="ps", bufs=4, space="PSUM") as ps,
            tc.tile_pool(name="esp", bufs=2) as esp,
        ):
            ebias_t = wp.tile([P, 1], f32, tag="ebias")
            nc.vector.memset(ebias_t[:], -3.0)
            gt_t = wp.tile([P, CO, C], f8, tag="gt")
            w2t_t = wp.tile([P, CO, C], f8, tag="w2t")
            ones_t = wp.tile([P, 2, P], f8, tag="ones")
            zbias_t = wp.tile([P, 1], f32, tag="zbias")
            nc.vector.memset(zbias_t[:], 0.0)
            x8_tiles = [
                wp.tile([P, CO, NB, 512], f8, tag=f"x8_{b}", name=f"x8_{b}")
                for b in range(BPC)
            ]
            xr_tiles = [
                wp.tile([P, CO, NB, 512], bf16, tag=f"xr_{b}", name=f"xr_{b}")
                for b in range(BPC)
            ]
            gt_r = gt_d.rearrange("(ci p) o -> p ci o", p=P)
            w2t_r = w2t_d.rearrange("(ci p) o -> p ci o", p=P)
            x8_r = [
                x8_d[b].rearrange("(ci p) (nb n) -> p ci nb n", p=P, nb=NB)
                for b in range(BPC)
            ]
            xr_r = [
                xr_d[b].rearrange("(ci p) (nb n) -> p ci nb n", p=P, nb=NB)
                for b in range(BPC)
            ]
            nc.sync.dma_start(gt_t[:], gt_r)
            for b in range(BPC):
                nc.sync.dma_start(x8_tiles[b][:], x8_r[b])
            nc.sync.dma_start(w2t_t[:], w2t_r)
            nc.sync.dma_start(ones_t[:], ones_d)
            for b in range(BPC):
                nc.sync.dma_start(xr_tiles[b][:], xr_r[b])

            def emit_A(b):
                """kp = G x (channel-major) and vp = (W2 x)^T (token-major)."""
                x8_t = x8_tiles[b]
                kp = kpp.tile([P, CO, NB, 512], f8, tag="kp")
                for oc in range(CO):
                    pt = ps.tile([P, NB, 512], f32, tag="ps", name="kp_pt")
                    for nb in range(NB):
                        for cip in range(0, CO, 2):
                            nc.tensor.matmul(
                                pt[:, nb],
                                gt_t[:, cip:cip + 2, oc * P:(oc + 1) * P],
                                x8_t[:, cip:cip + 2, nb],
                                start=(cip == 0), stop=(cip == CO - 2),
                                perf_mode=DR,
                            )
                    nc.scalar.activation(kp[:, oc], pt[:], Ident,
                                         bias=zbias_t[:])
                vp = vpp.tile([P, NT, C], f8, tag="vp")
                for t8p in range(0, NT, 2):
                    pt = ps.tile([P, 2, 512], f32, tag="ps", name="vp_pt")
                    for s in range(2):
                        t8 = t8p + s
                        for cip in range(0, CO, 2):
                            nc.tensor.matmul(
                                pt[:, s],
                                x8_t[:, cip:cip + 2, t8 // 4,
                                     (t8 % 4) * P:(t8 % 4 + 1) * P],
                                w2t_t[:, cip:cip + 2, :],
                                start=(cip == 0), stop=(cip == CO - 2),
                                perf_mode=DR,
                            )
                    nc.vector.tensor_copy(vp[:, t8p:t8p + 2, :], pt[:])
                return kp, vp

            def emit_B(b, kp):
                """s = kp^T x ; est = exp(s/sqrt(C)-3) fp8. The chunk sum
                esum accumulates on DVE as est tiles land (keeps the den
                work off the bottleneck PE); the cross-partition reduction
                is two cheap plain-fp8 ones-matmuls emitted by the caller."""
                x8_t = x8_tiles[b]
                est = epp.tile([P, NT, NB, 512], f8, tag="est")
                esum = esp.tile([P, 2, NB, 512], f32, tag="esum")
                esum8 = esp.tile([P, 2, NB, 512], f8, tag="esum8")
                for jc in range(NT):
                    pt = ps.tile([P, NB, 512], f32, tag="ps", name="sc_pt")
                    for ib in range(NB):
                        for ocp in range(0, CO, 2):
                            nc.tensor.matmul(
                                pt[:, ib],
                                kp[:, ocp:ocp + 2, jc // 4,
                                   (jc % 4) * P:(jc % 4 + 1) * P],
                                x8_t[:, ocp:ocp + 2, ib],
                                start=(ocp == 0), stop=(ocp == CO - 2),
                                perf_mode=DR,
                            )
                    nc.scalar.activation(est[:, jc], pt[:], Exp,
                                         bias=ebias_t[:], scale=inv_sqrt_c)
                    # two half-range chunk sums (fp8-safe magnitudes), each
                    # finishing in an fp8 tile for the ones-DoubleRow den
                    h, r = jc // 4, jc % 4
                    if r == 1:
                        nc.vector.tensor_add(esum[:, h], est[:, jc - 1],
                                             est[:, jc])
                    elif r == 2:
                        nc.vector.tensor_add(esum[:, h], esum[:, h],
                                             est[:, jc])
                    elif r == 3:
                        nc.vector.tensor_add(esum8[:, h], esum[:, h],
                                             est[:, jc])
                return est, esum8

            def emit_C(b, vp, est, esum8):
                """out_raw = vp^T est ; y = out_raw*recip + x' -> DRAM.
                den = cross-partition sum of esum8 via plain-fp8 ones
                matmuls, slotted between the first out_raw groups."""
                xr_t = xr_tiles[b]
                # den + recip first: esum8 is long since ready (the DVE
                # chain finished during A(t+1)), so recip is in flight
                # before the first out_raw group needs it
                recip = rpp.tile([P, NB, 512], f32, tag="recip")
                den_pt = ps.tile([P, NB, 512], f32, tag="ps", name="den_pt")
                for ib in range(NB):
                    nc.tensor.matmul(den_pt[:, ib], ones_t[:],
                                     esum8[:, :, ib, :], start=True,
                                     stop=True, perf_mode=DR)
                nc.vector.reciprocal(recip[:], den_pt[:])
                for cc in range(CO):
                    pt = ps.tile([P, NB, 512], f32, tag="ps", name="or_pt")
                    for ib in range(NB):
                        for jcp in range(0, NT, 2):
                            nc.tensor.matmul(
                                pt[:, ib],
                                vp[:, jcp:jcp + 2, cc * P:(cc + 1) * P],
                                est[:, jcp:jcp + 2, ib, :],
                                start=(jcp == 0), stop=(jcp == NT - 2),
                                perf_mode=DR,
                            )
                    t = tp.tile([P, NB, 512], bf16, tag="t")
                    nc.vector.tensor_mul(t[:], pt[:], recip[:])
                    # all-bf16 SBUF add on the otherwise idle GPSIMD engine
                    yt = yp.tile([P, NB, 512], bf16, tag="y")
                    nc.gpsimd.tensor_add(yt[:], t[:], xr_t[:, cc])
                    nc.sync.dma_start(
                        y_d[b, cc * P:(cc + 1) * P, :], yt[:])

            items = [i for _ in range(reps) for i in range(BPC)]
            kp_vp = emit_A(items[0])
            for ti, b in enumerate(items):
                kp, vp = kp_vp
                est, esum8 = emit_B(b, kp)
                if ti + 1 < len(items):
                    kp_vp = emit_A(items[ti + 1])
                emit_C(b, vp, est, esum8)
    nc.compile()
    return nc


def _prep_inputs(inputs):
    f8np = mybir.dt.np(mybir.dt.float8e4)
    bf16np = mybir.dt.np(mybir.dt.bfloat16)

    def q8(a):
        return np.clip(a, -240.0, 240.0).astype(f8np)

    x = np.asarray(inputs["x"], np.float32).reshape(B, C, N)
    wq = np.asarray(inputs["wq"], np.float64)
    wk = np.asarray(inputs["wk"], np.float64)
    wv = np.asarray(inputs["wv"], np.float64)
    wo = np.asarray(inputs["wo"], np.float64)
    bv = np.asarray(inputs["bv"], np.float64)
    bo = np.asarray(inputs["bo"], np.float64)

    G = (wq.T @ wk).astype(np.float32)      # s[j,i] = (G x_j) . x_i
    W2 = (wo @ wv).astype(np.float32)       # v'_j = W2 x_j
    bo_eff = (bo + wo @ bv).astype(np.float32)

    # lhsT layouts: gt[ci*P+p, o] = G[o, ci*P+p]; same for W2.
    shared = {
        "gt": np.ascontiguousarray(q8(G.T)),
        "w2t": np.ascontiguousarray(q8(W2.T)),
        "ones": np.ones((P, 2, P), f8np),
    }
    xr = (x + bo_eff[None, :, None]).astype(bf16np)
    in_maps = [
        {
            **shared,
            "x8": np.ascontiguousarray(q8(x[i * BPC:(i + 1) * BPC])),
            "xr": np.ascontiguousarray(xr[i * BPC:(i + 1) * BPC]),
        }
        for i in range(NCORES)
    ]
    return in_maps


def _make_axon_runner(nc):
    """Cached jitted shard_map runner for the axon/PJRT path."""
    import jax
    from jax.sharding import Mesh, NamedSharding, PartitionSpec

    import warnings

    with warnings.catch_warnings():
        warnings.simplefilter("ignore")
        from jax.experimental.shard_map import shard_map

    import concourse.bass2jax as b2j

    b2j.install_neuronx_cc_hook()
    partition_name = nc.partition_id_tensor.name if nc.partition_id_tensor else None
    in_names, out_names, out_avals = [], [], []
    for alloc in nc.m.functions[0].allocations:
        if not isinstance(alloc, mybir.MemoryLocationSet):
            continue
        name = alloc.memorylocations[0].name
        if alloc.kind == "ExternalInput":
            if name != partition_name:
                in_names.append(name)
        elif alloc.kind == "ExternalOutput":
            out_names.append(name)
            out_avals.append(
                jax.core.ShapedArray(tuple(alloc.tensor_shape),
                                     mybir.dt.np(alloc.dtype)))
    n_params = len(in_names)
    bind_in_names = list(in_names) + list(out_names)
    if partition_name is not None:
        bind_in_names.append(partition_name)

    def _body(*args):
        operands = list(args)
        if partition_name is not None:
            operands.append(b2j.partition_id_tensor())
        return tuple(b2j._bass_exec_p.bind(
            *operands,
            out_avals=tuple(out_avals),
            in_names=tuple(bind_in_names),
            out_names=tuple(out_names),
            lowering_input_output_aliases=(),
            sim_require_finite=True,
            sim_require_nnan=True,
            nc=nc,
        ))

    devices = jax.devices()[:NCORES]
    mesh = Mesh(np.asarray(devices), ("core",))
    n_outs = len(out_avals)
    fn = jax.jit(
        shard_map(_body, mesh=mesh,
                  in_specs=(PartitionSpec("core"),) * (n_params + n_outs),
                  out_specs=(PartitionSpec("core"),) * n_outs,
                  check_rep=False),
        keep_unused=True,
    )
    sharding = NamedSharding(mesh, PartitionSpec("core"))
    dev_zeros = [
        jax.device_put(
            np.zeros((NCORES * a.shape[0], *a.shape[1:]), a.dtype), sharding)
        for a in out_avals
    ]

    def run(in_maps):
        concat_in = [
            np.concatenate([np.asarray(m[nm]) for m in in_maps], axis=0)
            for nm in in_names
        ]
        dev_in = [jax.device_put(a, sharding) for a in concat_in]
        outs = fn(*dev_in, *dev_zeros)
        return np.asarray(outs[0])

    return run


def kernel(**inputs) -> np.ndarray:
    if "nc" not in _CACHE:
        _CACHE["nc"] = _build()
    nc = _CACHE["nc"]
    in_maps = _prep_inputs(inputs)

    from concourse._compat import axon_active

    if axon_active():
        if "runner" not in _CACHE:
            _CACHE["runner"] = _make_axon_runner(nc)
        y = _CACHE["runner"](in_maps).reshape(B, C, N)
    else:
        results = bass_utils.run_bass_kernel_spmd(
            nc, in_maps, core_ids=list(range(NCORES))).results
        y = np.concatenate([r["y"] for r in results], axis=0).reshape(B, C, N)
    return y.reshape(B, C, H, W).astype(np.float32)
